# revision 6
# baseline (speedup 1.0000x reference)
"""Fused attention block (QKV proj -> softmax attention -> out proj -> residual+LN)
for B=4, S=2048, D=512, H=8, DH=64 on 8 TRN2 NeuronCores.

Sharding: token-parallel. Core c handles batch b=c//2, query tokens
[(c%2)*1024, (c%2+1)*1024). Each core redundantly computes K/V for its
batch's full 2048-token sequence, runs flash-style attention on-chip, and
writes its disjoint 1024x512 output slice. Zero collectives.

v2 design (vs the bf16 baseline):
- All projections (Q/K/V/O) and the QK^T scores run as fp8e4m3 DoubleRow
  matmuls (0.5 PE cycles/row instead of 1.0). Host pre-scales x and the
  weights by 4 (16 for Wo) so fp8 values sit in the normal range; the 16x
  on Q/K cancels in the exp scale (0.125/256), the 16x on V cancels against
  the denominator, and the 256x on the out-proj is divided out in the
  residual-add (scalar_tensor_tensor with scalar=1/256).
- DoubleRow needs both contraction k-tiles on the same partitions, so K^T
  and Q^T are restaged into [32, head, 2, tok] fp8 tiles via small
  partition-remap DMAs issued from the (idle) GpSimd software DGE.
- raw ctx is stashed in f32 (no bf16 den casts); denominators are inverted
  per (head, half) with the fast approximate DVE reciprocal and broadcast
  across 64 partitions with a K=1 f32r ones-matmul.
- Out-proj runs in three phases (heads 0-3 after h3, 4-5 after h5, 6-7 in
  the tail) so the post-last-exp tail is ~8 x (1 DR matmul + add + LN).
- The LN final scale-shift runs on the Scalar engine (Identity activation,
  scale=rstd, bias=-mu*rstd) which is idle after the last exp.
- exp stays on the Scalar engine: 128 x [128,1024] EXP is the ~171us floor
  this schedule is built to keep dense.
"""

import os
import sys

import numpy as np

for _p in ("/opt/trn_rl_repo",):
    if os.path.isdir(_p) and _p not in sys.path:
        sys.path.insert(0, _p)

import ml_dtypes

import concourse.bacc as bacc
import concourse.bass as bass
import concourse.tile as tile
from concourse import mybir
from concourse.bass_utils import run_bass_kernel_spmd

BF16 = mybir.dt.bfloat16
F32 = mybir.dt.float32
F32R = mybir.dt.float32r
F8 = mybir.dt.float8e4
AF = mybir.ActivationFunctionType
ALU = mybir.AluOpType
DR = mybir.MatmulPerfMode.DoubleRow

P = 128        # partitions
D = 512        # hidden dim
DH = 64        # head dim
H = 8          # heads
S = 2048       # tokens per batch element
TQ = 1024      # query tokens per core
B = 4
NCORES = 8
EPS = 1e-5

SX = 4.0       # host scale on x
SW = 4.0       # host scale on Wq/Wk/Wv (Wo gets SX*SW)
SQK = SX * SW                  # scale sitting on Q and K (and V)
EXP_SCALE = 0.125 / (SQK * SQK)  # undo Q*K scaling inside the exp
INV_O = 1.0 / (SQK * SQK)        # undo ctx16 * Wo16 scaling at the y-add

TRACE = False
LAST_RESULTS = None
_NC_CACHE = None


def _build():
    nc = bacc.Bacc()

    # x permuted (local queries first), scaled by SX, fp8, laid out
    # [64, t4, j, tok]: element (d, t4, j, t) = SX * xperm[512*t4+t, 64*j+d]
    xt8 = nc.declare_dram_parameter("xt8", [DH, 4 * 8 * D], F8, isOutput=False)
    # xres[p, i*512+d] = xloc[i*128+p, d] + bo[d]
    xres = nc.declare_dram_parameter("xres", [P, 8 * D], BF16, isOutput=False)
    # w*8[d, j, m] = SW * W[m, 64j+d]  (wo8 uses SQK * Wo)
    wq8 = nc.declare_dram_parameter("wq8", [DH, 8 * D], F8, isOutput=False)
    wk8 = nc.declare_dram_parameter("wk8", [DH, 8 * D], F8, isOutput=False)
    wv8 = nc.declare_dram_parameter("wv8", [DH, 8 * D], F8, isOutput=False)
    wo8 = nc.declare_dram_parameter("wo8", [DH, 8 * D], F8, isOutput=False)
    bqp = nc.declare_dram_parameter("bq", [P, 4], F32, isOutput=False)   # 16*bq
    bkp = nc.declare_dram_parameter("bk", [P, 4], F32, isOutput=False)
    bvp = nc.declare_dram_parameter("bv", [P, D], BF16, isOutput=False)  # 16*bv bcast
    # out[p, i*512+d] = LN(y)[i*128+p, d]  (gamma/beta applied on host)
    outp = nc.declare_dram_parameter("out", [P, 8 * D], BF16, isOutput=True)

    with tile.TileContext(nc) as tc:
        with (
            tc.tile_pool(name="big", bufs=1) as big,
            tc.tile_pool(name="work", bufs=3) as work,
            tc.tile_pool(name="ps_st", bufs=2, space="PSUM") as ps_st,
            tc.tile_pool(name="ps_ctx", bufs=2, space="PSUM") as ps_ctx,
            tc.tile_pool(name="ps_mm", bufs=2, space="PSUM") as ps_mm,
        ):
            # ---------------- SBUF tiles ----------------
            xt_sb = big.tile([DH, 4, 8, D], F8)     # [d, t4, j, tok]
            wk_sb = big.tile([DH, 8, D], F8)
            wq_sb = big.tile([DH, 8, D], F8)
            wv_sb = big.tile([DH, 8, D], F8)
            wo_sb = big.tile([DH, 8, D], F8)
            bq_sb = big.tile([P, 4], F32)
            bk_sb = big.tile([P, 4], F32)
            bv_sb = big.tile([P, D], BF16)
            xres_sb = big.tile([P, 8, D], BF16)

            kt8 = big.tile([P, 4, S], F8)           # K^T staging [dim, m, tok]
            qt8 = big.tile([P, 4, TQ], F8)
            ktdr = big.tile([32, H, 2, S], F8)      # DR layout [p, h, ktile, tok]
            qtdr = big.tile([32, H, 2, TQ], F8)
            vaug = big.tile([P, 16, H * 65], BF16)  # V16 + ones col per head
            rawc = big.tile([65, 16, D], BF16)      # unnormalized ctx + den row
            den_rec = big.tile([1, 4, D], F32)      # den -> 1/den ring (partition 0)
            ctx8 = big.tile([DH, H, TQ], F8)        # normalized ctx * 16
            y_all = big.tile([P, 8, D], F32)
            mv_all = big.tile([P, 8, 2], F32)
            rstd_all = big.tile([P, 8], F32)
            negb_all = big.tile([P, 8], F32)
            ones_sb = big.tile([1, DH], F32)
            eps_sb = big.tile([P, 1], F32)

            # ---------------- loads (priority order) ----------------
            nc.sync.dma_start(out=bq_sb[:, :], in_=bqp[:, :])
            nc.sync.dma_start(out=bk_sb[:, :], in_=bkp[:, :])
            nc.sync.dma_start(out=wk_sb[:, :, :], in_=wk8[:, :])
            nc.sync.dma_start(
                out=xt_sb[:, 0, :, :], in_=xt8[:, 0:8 * D]
            )
            nc.sync.dma_start(out=wq_sb[:, :, :], in_=wq8[:, :])
            nc.sync.dma_start(
                out=xt_sb[:, 1, :, :], in_=xt8[:, 8 * D:16 * D]
            )
            nc.sync.dma_start(out=wv_sb[:, :, :], in_=wv8[:, :])
            nc.sync.dma_start(out=bv_sb[:, :], in_=bvp[:, :])
            nc.sync.dma_start(
                out=xt_sb[:, 2, :, :], in_=xt8[:, 16 * D:24 * D]
            )
            nc.sync.dma_start(
                out=xt_sb[:, 3, :, :], in_=xt8[:, 24 * D:32 * D]
            )
            nc.gpsimd.dma_start(out=wo_sb[:, :, :], in_=wo8[:, :])
            nc.gpsimd.dma_start(out=xres_sb[:, :, :], in_=xres[:, :])

            nc.vector.memset(ones_sb[:, :], 1.0)
            nc.vector.memset(eps_sb[:, :], EPS)
            nc.vector.memset(
                vaug[:, :, :].rearrange("p c (h e) -> p c h e", e=65)[:, :, :, 64:65],
                1.0,
            )

            # ---------------- projection emitters ----------------
            def k_proj(m, t4):
                ps = ps_mm.tile([P, D], F32, tag="mm", name="ps_k")
                for c in range(4):
                    nc.tensor.matmul(
                        ps[:, :],
                        lhsT=wk_sb[:, 2 * c:2 * c + 2, m * P:(m + 1) * P],
                        rhs=xt_sb[:, t4, 2 * c:2 * c + 2, :],
                        start=(c == 0),
                        stop=(c == 3),
                        perf_mode=DR,
                    )
                sl = slice(t4 * D, (t4 + 1) * D)
                nc.vector.tensor_scalar_add(kt8[:, m, sl], ps[:, :], bk_sb[:, m:m + 1])
                for q in range(4):
                    nc.gpsimd.dma_start(
                        out=ktdr[0:32, 2 * m + q // 2, q % 2, sl],
                        in_=kt8[32 * q:32 * (q + 1), m, sl],
                    )

            def q_proj(m, t2):
                ps = ps_mm.tile([P, D], F32, tag="mm", name="ps_q")
                for c in range(4):
                    nc.tensor.matmul(
                        ps[:, :],
                        lhsT=wq_sb[:, 2 * c:2 * c + 2, m * P:(m + 1) * P],
                        rhs=xt_sb[:, t2, 2 * c:2 * c + 2, :],
                        start=(c == 0),
                        stop=(c == 3),
                        perf_mode=DR,
                    )
                sl = slice(t2 * D, (t2 + 1) * D)
                nc.vector.tensor_scalar_add(qt8[:, m, sl], ps[:, :], bq_sb[:, m:m + 1])
                for q in range(4):
                    nc.sync.dma_start(
                        out=qtdr[0:32, 2 * m + q // 2, q % 2, sl],
                        in_=qt8[32 * q:32 * (q + 1), m, sl],
                    )

            def v_proj(t16):
                t4, tt = t16 // 4, t16 % 4
                ps = ps_mm.tile([P, D], F32, tag="mm", name="ps_v")
                for c in range(4):
                    nc.tensor.matmul(
                        ps[:, :],
                        lhsT=xt_sb[:, t4, 2 * c:2 * c + 2, tt * P:(tt + 1) * P],
                        rhs=wv_sb[:, 2 * c:2 * c + 2, :],
                        start=(c == 0),
                        stop=(c == 3),
                        perf_mode=DR,
                    )
                nc.vector.tensor_add(
                    out=vaug[:, t16, :].rearrange("p (h e) -> p h e", e=65)[:, :, 0:64],
                    in0=ps[:, :].rearrange("p (h e) -> p h e", e=64),
                    in1=bv_sb[:, :].rearrange("p (h e) -> p h e", e=64),
                )

            # normalize slot s = 2h + half: broadcast 1/den across 64
            # partitions (K=1 f32r matmul) and scale the raw ctx into ctx8
            def norm_slot(s):
                h, qt2 = s // 2, s % 2
                rb = ps_mm.tile([P, D], F32, tag="mm", name="ps_rb")
                nc.tensor.matmul(
                    rb[0:DH, :],
                    lhsT=ones_sb[0:1, :],
                    rhs=den_rec[0:1, s % 4, :],
                    start=True,
                    stop=True,
                )
                nc.vector.tensor_mul(
                    out=ctx8[:, h, qt2 * D:(qt2 + 1) * D],
                    in0=rawc[0:DH, s, :],
                    in1=rb[0:DH, :],
                )

            # out-projection phases: A = heads 0-3 (+xres), B = heads 4-5,
            # C = heads 6-7 (+ LayerNorm + store), accumulated via
            # y += ps/256 on the Vector engine.
            def phase_a(t8):
                ps = ps_mm.tile([P, D], F32, tag="mm", name="ps_oa")
                for c in range(2):
                    nc.tensor.matmul(
                        ps[:, :],
                        lhsT=ctx8[:, 2 * c:2 * c + 2, t8 * P:(t8 + 1) * P],
                        rhs=wo_sb[:, 2 * c:2 * c + 2, :],
                        start=(c == 0),
                        stop=(c == 1),
                        perf_mode=DR,
                    )
                nc.vector.scalar_tensor_tensor(
                    out=y_all[:, t8, :], in0=ps[:, :], scalar=INV_O,
                    in1=xres_sb[:, t8, :], op0=ALU.mult, op1=ALU.add,
                )

            def phase_b(t8):
                ps = ps_mm.tile([P, D], F32, tag="mm", name="ps_ob")
                nc.tensor.matmul(
                    ps[:, :],
                    lhsT=ctx8[:, 4:6, t8 * P:(t8 + 1) * P],
                    rhs=wo_sb[:, 4:6, :],
                    start=True,
                    stop=True,
                    perf_mode=DR,
                )
                nc.vector.scalar_tensor_tensor(
                    out=y_all[:, t8, :], in0=ps[:, :], scalar=INV_O,
                    in1=y_all[:, t8, :], op0=ALU.mult, op1=ALU.add,
                )

            def phase_c(t8):
                ps = ps_mm.tile([P, D], F32, tag="mm", name="ps_oc")
                nc.tensor.matmul(
                    ps[:, :],
                    lhsT=ctx8[:, 6:8, t8 * P:(t8 + 1) * P],
                    rhs=wo_sb[:, 6:8, :],
                    start=True,
                    stop=True,
                    perf_mode=DR,
                )
                nc.vector.scalar_tensor_tensor(
                    out=y_all[:, t8, :], in0=ps[:, :], scalar=INV_O,
                    in1=y_all[:, t8, :], op0=ALU.mult, op1=ALU.add,
                )
                stt = work.tile([P, 6], F32, tag="bnst")
                nc.vector.bn_stats(out=stt[:, :], in_=y_all[:, t8, :])
                nc.vector.bn_aggr(out=mv_all[:, t8, :], in_=stt[:, :])
                std1 = work.tile([P, 1], F32, tag="std1")
                nc.scalar.activation(
                    out=std1[:, :], in_=mv_all[:, t8, 1:2], func=AF.Sqrt,
                    bias=eps_sb[:, :], scale=1.0,
                )
                nc.vector.reciprocal(rstd_all[:, t8:t8 + 1], std1[:, :])
                nc.vector.tensor_scalar(
                    out=negb_all[:, t8:t8 + 1], in0=mv_all[:, t8, 0:1],
                    scalar1=rstd_all[:, t8:t8 + 1], scalar2=-1.0,
                    op0=ALU.mult, op1=ALU.mult,
                )
                fin = work.tile([P, D], BF16, tag="fin")
                nc.scalar.activation(
                    out=fin[:, :], in_=y_all[:, t8, :], func=AF.Identity,
                    bias=negb_all[:, t8:t8 + 1], scale=rstd_all[:, t8:t8 + 1],
                )
                nc.sync.dma_start(out=outp[:, t8 * D:(t8 + 1) * D], in_=fin[:, :])

            # ---------------- interleave schedule ----------------
            # ramp: K/Q head-pair 0 for the first token halves
            k_proj(0, 0)
            q_proj(0, 0)
            k_proj(0, 1)
            q_proj(0, 1)

            inter = {h: {} for h in range(H)}

            def put(h, kc, u):
                inter[h].setdefault(kc, []).append(u)

            # head 0: V just-in-time + rest of K0
            put(0, 0, lambda: v_proj(0))
            put(0, 0, lambda: v_proj(1))
            for t in range(1, 15):
                put(0, t, lambda t=t: v_proj(t + 1))
            put(0, 5, lambda: k_proj(0, 2))
            put(0, 9, lambda: k_proj(0, 3))
            # head 1: m=1 units (heads 2,3)
            for i in range(4):
                put(1, 2 * i, lambda i=i: k_proj(1, i))
            put(1, 8, lambda: q_proj(1, 0))
            put(1, 10, lambda: q_proj(1, 1))
            # heads 2-3: m=2 units (heads 4,5)
            put(2, 0, lambda: k_proj(2, 0))
            put(2, 5, lambda: k_proj(2, 1))
            put(2, 10, lambda: k_proj(2, 2))
            put(3, 0, lambda: k_proj(2, 3))
            put(3, 5, lambda: q_proj(2, 0))
            put(3, 10, lambda: q_proj(2, 1))
            # heads 4-5: m=3 units (heads 6,7) + out-proj phase A
            put(4, 0, lambda: k_proj(3, 0))
            put(4, 4, lambda: k_proj(3, 1))
            put(4, 5, lambda: k_proj(3, 2))
            put(4, 9, lambda: k_proj(3, 3))
            put(5, 5, lambda: q_proj(3, 0))
            put(5, 10, lambda: q_proj(3, 1))
            for i, kc in enumerate((6, 8, 10, 12, 14)):
                put(4, kc, lambda i=i: phase_a(i))
            for i, kc in enumerate((0, 2, 4)):
                put(5, kc, lambda i=i: phase_a(5 + i))
            # phase B in head 6
            for i, kc in enumerate(range(5, 13)):
                put(6, kc, lambda i=i: phase_b(i))
            # deferred normalization of the previous head's two slots
            for h in range(1, 8):
                put(h, 1, lambda s=2 * (h - 1): norm_slot(s))
                put(h, 3, lambda s=2 * (h - 1) + 1: norm_slot(s))

            # ---------------- attention ----------------
            for h in range(H):
                cx0 = ps_ctx.tile([65, D], F32, tag="cx")
                cx1 = ps_ctx.tile([65, D], F32, tag="cx")
                for kc in range(16):
                    for u in inter[h].get(kc, []):
                        u()
                    st = ps_st.tile([P, TQ], F32, tag="st")
                    nc.tensor.matmul(
                        st[:, 0:D],
                        lhsT=ktdr[0:32, h, :, kc * P:(kc + 1) * P],
                        rhs=qtdr[0:32, h, :, 0:D],
                        start=True,
                        stop=True,
                        perf_mode=DR,
                    )
                    nc.tensor.matmul(
                        st[:, D:TQ],
                        lhsT=ktdr[0:32, h, :, kc * P:(kc + 1) * P],
                        rhs=qtdr[0:32, h, :, D:TQ],
                        start=True,
                        stop=True,
                        perf_mode=DR,
                    )
                    pr = work.tile([P, TQ], BF16, tag="probs")
                    nc.scalar.activation(
                        out=pr[:, :], in_=st[:, :], func=AF.Exp, scale=EXP_SCALE
                    )
                    vh = vaug[:, kc, h * 65:(h + 1) * 65]
                    nc.tensor.matmul(
                        cx0[:, :], lhsT=vh, rhs=pr[:, 0:D],
                        start=(kc == 0), stop=(kc == 15),
                    )
                    nc.tensor.matmul(
                        cx1[:, :], lhsT=vh, rhs=pr[:, D:TQ],
                        start=(kc == 0), stop=(kc == 15),
                    )
                # stash raw ctx (+ den row 64), gather den to partition 0
                # (casting DMA), invert in place
                for qt2, cx in ((0, cx0), (1, cx1)):
                    s = 2 * h + qt2
                    nc.vector.tensor_copy(rawc[0:65, s, :], cx[0:65, :])
                    nc.gpsimd.dma_start(
                        out=den_rec[0:1, s % 4, :], in_=rawc[64:65, s, :]
                    )
                    nc.vector.reciprocal_approx_fast(
                        den_rec[0:1, s % 4, :], den_rec[0:1, s % 4, :]
                    )

            # ---------------- tail ----------------
            norm_slot(14)
            norm_slot(15)
            for t8 in range(8):
                phase_c(t8)

    nc.compile()
    return nc


def _get_nc():
    global _NC_CACHE
    if _NC_CACHE is None:
        _NC_CACHE = _build()
    return _NC_CACHE


def _prep_in_maps(x, Wq, bq, Wk, bk, Wv, bv, Wo, bo):
    bf = ml_dtypes.bfloat16
    f8 = ml_dtypes.float8_e4m3
    x = np.asarray(x, np.float32)
    bo = np.asarray(bo, np.float32)

    def wprep(w, s):
        # [64, 8, 512]: (d, j, m) = s * W[m, 64j+d]
        a = (np.asarray(w, np.float32).T * s).reshape(8, DH, D)
        return np.ascontiguousarray(a.transpose(1, 0, 2).reshape(DH, 8 * D)).astype(f8)

    wq8 = wprep(Wq, SW)
    wk8 = wprep(Wk, SW)
    wv8 = wprep(Wv, SW)
    wo8 = wprep(Wo, SQK)
    bq16 = np.ascontiguousarray(
        (np.asarray(bq, np.float32) * SQK).reshape(4, P).T)
    bk16 = np.ascontiguousarray(
        (np.asarray(bk, np.float32) * SQK).reshape(4, P).T)
    bv16 = np.ascontiguousarray(np.broadcast_to(
        (np.asarray(bv, np.float32) * SQK)[None, :], (P, D))).astype(bf)

    in_maps = []
    for c in range(NCORES):
        b = c // 2
        par = c % 2
        xb = x[b]
        xloc = xb[par * TQ:(par + 1) * TQ]
        xoth = xb[(1 - par) * TQ:(2 - par) * TQ]
        xperm = np.concatenate([xloc, xoth], axis=0)   # local queries first
        # xt8[d, t4, j, t] = SX * xperm[512*t4+t, 64j+d]
        a = (xperm.T * SX).reshape(8, DH, 4, D).transpose(1, 2, 0, 3)
        xt8_n = np.ascontiguousarray(a.reshape(DH, 4 * 8 * D)).astype(f8)
        # xres[p, i*512+d] = xloc[i*128+p, d] + bo[d]
        xr = (xloc + bo[None, :]).reshape(8, P, D).transpose(1, 0, 2)
        xres_n = np.ascontiguousarray(xr.reshape(P, 8 * D)).astype(bf)
        in_maps.append({
            "xt8": xt8_n, "xres": xres_n,
            "wq8": wq8, "wk8": wk8, "wv8": wv8, "wo8": wo8,
            "bq": bq16, "bk": bk16, "bv": bv16,
        })
    return in_maps


def kernel(x, Wq, bq, Wk, bk, Wv, bv, Wo, bo, gamma, beta):
    global LAST_RESULTS
    in_maps = _prep_in_maps(x, Wq, bq, Wk, bk, Wv, bv, Wo, bo)

    nc = _get_nc()
    res = run_bass_kernel_spmd(nc, in_maps, core_ids=list(range(NCORES)), trace=TRACE)
    LAST_RESULTS = res

    outf = np.empty((B, S, D), np.float32)
    for c in range(NCORES):
        b = c // 2
        par = c % 2
        o = np.asarray(res.results[c]["out"], dtype=np.float32)
        # out[p, i*512+d] -> tokens
        o = o.reshape(P, 8, D).transpose(1, 0, 2).reshape(TQ, D)
        outf[b, par * TQ:(par + 1) * TQ, :] = o
    gm = np.asarray(gamma, np.float32)[None, None, :]
    bt = np.asarray(beta, np.float32)[None, None, :]
    return outf * gm + bt


# revision 7
# speedup vs baseline: 1.8350x; 1.8350x over previous
"""Fused attention block (QKV proj -> softmax attention -> out proj -> residual+LN)
for B=4, S=2048, D=512, H=8, DH=64 on 8 TRN2 NeuronCores.

Sharding: token-parallel. Core c handles batch b=c//2, query tokens
[(c%2)*1024, (c%2+1)*1024). Each core redundantly computes K/V for its
batch's full 2048-token sequence, runs flash-style attention on-chip, and
writes its disjoint 1024x512 output slice. Zero collectives.

v3 (vs the 272us baseline):
- all-bf16 matmuls (fp8 DoubleRow measured no faster than bf16 on hw, and
  the extra staging stalled the PE out of its fast pstate)
- denominator path: raw ctx stashed bf16, den row gathered to partition 0
  via a casting GpSimd DMA, inverted in place with the fast approx DVE
  reciprocal, cast to bf16, broadcast across 64 partitions with a K=1
  ones-matmul (no grouped gathers / f32 recip staging of the baseline)
- out-proj split in three phases (heads 0-3 after h3, head-pair 2 after
  h5, head-pair 3 in the tail) so the post-last-exp tail is short
- LN: bn_stats/aggr + exact reciprocal on DVE, sqrt on ACT, and the final
  (y-mu)*rstd as an Identity activation on the (then idle) Scalar engine
- xres/out use a [128, 8*512] layout so they move as one DMA each, issued
  from the idle GpSimd queue
"""

import os
import sys

import numpy as np

for _p in ("/opt/trn_rl_repo",):
    if os.path.isdir(_p) and _p not in sys.path:
        sys.path.insert(0, _p)

import ml_dtypes

import concourse.bacc as bacc
import concourse.bass as bass
import concourse.tile as tile
from concourse import mybir
from concourse.bass_utils import run_bass_kernel_spmd

BF16 = mybir.dt.bfloat16
F32 = mybir.dt.float32
AF = mybir.ActivationFunctionType
ALU = mybir.AluOpType

P = 128        # partitions
D = 512        # hidden dim
DH = 64        # head dim
H = 8          # heads
S = 2048       # tokens per batch element
TQ = 1024      # query tokens per core
B = 4
NCORES = 8
EPS = 1e-5

TRACE = False
LAST_RESULTS = None
_NC_CACHE = None


def _build():
    nc = bacc.Bacc()

    # x[b] permuted so local query tokens are first, then transposed: [D, S]
    xt = nc.declare_dram_parameter("xt", [D, S], BF16, isOutput=False)
    # xres[p, i*512+d] = xloc[i*128+p, d] + bo[d]
    xres = nc.declare_dram_parameter("xres", [P, 8 * D], BF16, isOutput=False)
    wqt = nc.declare_dram_parameter("wqt", [D, D], BF16, isOutput=False)     # Wq.T
    wkt = nc.declare_dram_parameter("wkt", [D, D], BF16, isOutput=False)
    wvt = nc.declare_dram_parameter("wvt", [D, D], BF16, isOutput=False)
    wot = nc.declare_dram_parameter("wot", [D, D], BF16, isOutput=False)
    bqp = nc.declare_dram_parameter("bq", [P, 4], F32, isOutput=False)   # bq.reshape(4,128).T
    bkp = nc.declare_dram_parameter("bk", [P, 4], F32, isOutput=False)
    bvp = nc.declare_dram_parameter("bv", [P, D], BF16, isOutput=False)  # host-broadcast
    # out[p, i*512+d] = LN(y)[i*128+p, d]  (gamma/beta applied on host)
    outp = nc.declare_dram_parameter("out", [P, 8 * D], BF16, isOutput=True)

    with tile.TileContext(nc) as tc:
        with (
            tc.tile_pool(name="big", bufs=1) as big,
            tc.tile_pool(name="work", bufs=4) as work,
            tc.tile_pool(name="ps_st", bufs=2, space="PSUM") as ps_st,
            tc.tile_pool(name="ps_ctx", bufs=2, space="PSUM") as ps_ctx,
            tc.tile_pool(name="ps_mm", bufs=2, space="PSUM") as ps_mm,
        ):
            # ---------------- loads (priority order) ----------------
            wk_sb = big.tile([P, 4, D], BF16)
            xt_sb = big.tile([P, 4, S], BF16)
            wq_sb = big.tile([P, 4, D], BF16)
            wv_sb = big.tile([P, 4, D], BF16)
            wo_sb = big.tile([P, 4, D], BF16)
            bq_sb = big.tile([P, 4], F32)
            bk_sb = big.tile([P, 4], F32)
            bv_sb = big.tile([P, D], BF16)
            xres_sb = big.tile([P, 8, D], BF16)
            nc.sync.dma_start(out=bq_sb[:, :], in_=bqp[:, :])
            nc.sync.dma_start(out=bk_sb[:, :], in_=bkp[:, :])
            # weights split by m-column so m=0 (head-pair 0) lands first
            for c in range(4):
                nc.sync.dma_start(out=wk_sb[:, c, 0:P], in_=wkt[c * P:(c + 1) * P, 0:P])
            # xt in [kc, t4] chunks, t4-major so K/Q of m=0 unblock first
            for t4 in range(2):
                for c in range(4):
                    nc.sync.dma_start(
                        out=xt_sb[:, c, t4 * D:(t4 + 1) * D],
                        in_=xt[c * P:(c + 1) * P, t4 * D:(t4 + 1) * D],
                    )
            for c in range(4):
                nc.sync.dma_start(out=wq_sb[:, c, 0:P], in_=wqt[c * P:(c + 1) * P, 0:P])
            nc.sync.dma_start(out=bv_sb[:, :], in_=bvp[:, :])
            for c in range(4):
                nc.sync.dma_start(out=wv_sb[:, c, :], in_=wvt[c * P:(c + 1) * P, :])
            for t4 in range(2, 4):
                for c in range(4):
                    nc.sync.dma_start(
                        out=xt_sb[:, c, t4 * D:(t4 + 1) * D],
                        in_=xt[c * P:(c + 1) * P, t4 * D:(t4 + 1) * D],
                    )
            for c in range(4):
                nc.sync.dma_start(out=wk_sb[:, c, P:4 * P], in_=wkt[c * P:(c + 1) * P, P:4 * P])
                nc.sync.dma_start(out=wq_sb[:, c, P:4 * P], in_=wqt[c * P:(c + 1) * P, P:4 * P])
            for c in range(4):
                nc.gpsimd.dma_start(out=wo_sb[:, c, :], in_=wot[c * P:(c + 1) * P, :])
            nc.gpsimd.dma_start(out=xres_sb[:, :, :], in_=xres[:, :])

            ones_sb = big.tile([1, DH], BF16)
            nc.vector.memset(ones_sb[:, :], 1.0)
            eps_sb = big.tile([P, 1], F32)
            nc.vector.memset(eps_sb[:, :], EPS)

            # V augmented with a ones column per head: [tok, (h, 64 dims + 1)]
            vaug = big.tile([P, 16, H * 65], BF16)
            nc.vector.memset(
                vaug[:, :, :].rearrange("p c (h e) -> p c h e", e=65)[:, :, :, 64:65],
                1.0,
            )

            qt_all = big.tile([P, 4, TQ], BF16)   # Q^T  [dq, tq]
            kt_all = big.tile([P, 4, S], BF16)    # K^T  [dk, t]
            qt_dup = big.tile([P, 4, TQ], BF16)   # partition-swapped copy of Q^T
            kt_dup = big.tile([P, 4, S], BF16)    # partition-swapped copy of K^T
            ctxT = big.tile([P, 4, TQ], BF16)     # normalized ctx^T [dv, tq]
            rawc = big.tile([65, 16, D], BF16)    # unnormalized ctx + den row
            den_rec = big.tile([1, 4, D], F32)    # den -> 1/den ring (partition 0)
            rec_c = big.tile([1, 4, D], BF16)     # 1/den in bf16 for the bcast mm
            y_all = big.tile([P, 8, D], F32)      # proj + residual
            mv_all = big.tile([P, 8, 2], F32)     # (mean, var) per token tile
            rstd_all = big.tile([P, 8], F32)
            negb_all = big.tile([P, 8], F32)

            # ---------------- projection emitters ----------------
            def k_proj(m, t4):
                ps = ps_mm.tile([P, D], F32, tag="mm", name="ps_k")
                for kc in range(4):
                    nc.tensor.matmul(
                        ps[:, :],
                        lhsT=wk_sb[:, kc, m * P:(m + 1) * P],
                        rhs=xt_sb[:, kc, t4 * D:(t4 + 1) * D],
                        start=(kc == 0),
                        stop=(kc == 3),
                    )
                nc.vector.tensor_scalar_add(
                    kt_all[:, m, t4 * D:(t4 + 1) * D], ps[:, :], bk_sb[:, m:m + 1]
                )
                sl = slice(t4 * D, (t4 + 1) * D)
                nc.sync.dma_start(out=kt_dup[64:128, m, sl], in_=kt_all[0:64, m, sl])
                nc.sync.dma_start(out=kt_dup[0:64, m, sl], in_=kt_all[64:128, m, sl])

            def q_proj(m, t2):
                ps = ps_mm.tile([P, D], F32, tag="mm", name="ps_q")
                for kc in range(4):
                    nc.tensor.matmul(
                        ps[:, :],
                        lhsT=wq_sb[:, kc, m * P:(m + 1) * P],
                        rhs=xt_sb[:, kc, t2 * D:(t2 + 1) * D],
                        start=(kc == 0),
                        stop=(kc == 3),
                    )
                nc.vector.tensor_scalar_add(
                    qt_all[:, m, t2 * D:(t2 + 1) * D], ps[:, :], bq_sb[:, m:m + 1]
                )
                sl = slice(t2 * D, (t2 + 1) * D)
                nc.sync.dma_start(out=qt_dup[64:128, m, sl], in_=qt_all[0:64, m, sl])
                nc.sync.dma_start(out=qt_dup[0:64, m, sl], in_=qt_all[64:128, m, sl])

            def v_proj(t16):
                ps = ps_mm.tile([P, D], F32, tag="mm", name="ps_v")
                for kc in range(4):
                    nc.tensor.matmul(
                        ps[:, :],
                        lhsT=xt_sb[:, kc, t16 * P:(t16 + 1) * P],
                        rhs=wv_sb[:, kc, :],
                        start=(kc == 0),
                        stop=(kc == 3),
                    )
                nc.vector.tensor_add(
                    out=vaug[:, t16, :].rearrange("p (h e) -> p h e", e=65)[:, :, 0:64],
                    in0=ps[:, :].rearrange("p (h e) -> p h e", e=64),
                    in1=bv_sb[:, :].rearrange("p (h e) -> p h e", e=64),
                )

            # normalize slot s = 2h + half: bf16 recip row, broadcast across
            # 64 partitions with a K=1 ones-matmul, scale raw ctx into ctxT
            def norm_slot(s):
                h, qt2 = s // 2, s % 2
                po = (h % 2) * 64
                chn = h // 2
                nc.vector.tensor_copy(rec_c[0:1, s % 4, :], den_rec[0:1, s % 4, :])
                rb = ps_mm.tile([P, D], F32, tag="mm", name="ps_rb")
                nc.tensor.matmul(
                    rb[0:DH, :],
                    lhsT=ones_sb[0:1, :],
                    rhs=rec_c[0:1, s % 4, :],
                    start=True,
                    stop=True,
                )
                nc.vector.tensor_mul(
                    out=ctxT[po:po + 64, chn, qt2 * D:(qt2 + 1) * D],
                    in0=rawc[0:DH, s, :],
                    in1=rb[0:DH, :],
                )

            # out-projection phases: A = chunks 0,1 (+xres), B = chunk 2,
            # C = chunk 3 (+ LayerNorm + store)
            def phase_a(t8):
                ps = ps_mm.tile([P, D], F32, tag="mm", name="ps_oa")
                for c in range(2):
                    nc.tensor.matmul(
                        ps[:, :],
                        lhsT=ctxT[:, c, t8 * P:(t8 + 1) * P],
                        rhs=wo_sb[:, c, :],
                        start=(c == 0),
                        stop=(c == 1),
                    )
                nc.vector.tensor_add(
                    out=y_all[:, t8, :], in0=ps[:, :], in1=xres_sb[:, t8, :]
                )

            def phase_b(t8):
                ps = ps_mm.tile([P, D], F32, tag="mm", name="ps_ob")
                nc.tensor.matmul(
                    ps[:, :],
                    lhsT=ctxT[:, 2, t8 * P:(t8 + 1) * P],
                    rhs=wo_sb[:, 2, :],
                    start=True,
                    stop=True,
                )
                nc.vector.tensor_add(
                    out=y_all[:, t8, :], in0=ps[:, :], in1=y_all[:, t8, :]
                )

            def phase_c(t8):
                ps = ps_mm.tile([P, D], F32, tag="mm", name="ps_oc")
                nc.tensor.matmul(
                    ps[:, :],
                    lhsT=ctxT[:, 3, t8 * P:(t8 + 1) * P],
                    rhs=wo_sb[:, 3, :],
                    start=True,
                    stop=True,
                )
                nc.vector.tensor_add(
                    out=y_all[:, t8, :], in0=ps[:, :], in1=y_all[:, t8, :]
                )
                stt = work.tile([P, 6], F32, tag="bnst")
                nc.vector.bn_stats(out=stt[:, :], in_=y_all[:, t8, :])
                nc.vector.bn_aggr(out=mv_all[:, t8, :], in_=stt[:, :])
                std1 = work.tile([P, 1], F32, tag="std1")
                nc.scalar.activation(
                    out=std1[:, :], in_=mv_all[:, t8, 1:2], func=AF.Sqrt,
                    bias=eps_sb[:, :], scale=1.0,
                )
                nc.vector.reciprocal(rstd_all[:, t8:t8 + 1], std1[:, :])
                nc.vector.tensor_scalar(
                    out=negb_all[:, t8:t8 + 1], in0=mv_all[:, t8, 0:1],
                    scalar1=rstd_all[:, t8:t8 + 1], scalar2=-1.0,
                    op0=ALU.mult, op1=ALU.mult,
                )
                fin = work.tile([P, D], BF16, tag="fin")
                nc.scalar.activation(
                    out=fin[:, :], in_=y_all[:, t8, :], func=AF.Identity,
                    bias=negb_all[:, t8:t8 + 1], scale=rstd_all[:, t8:t8 + 1],
                )
                nc.sync.dma_start(out=outp[:, t8 * D:(t8 + 1) * D], in_=fin[:, :])

            # ---------------- interleave schedule ----------------
            # ramp: K/Q head-pair 0 for the first token halves
            k_proj(0, 0)
            q_proj(0, 0)
            k_proj(0, 1)
            q_proj(0, 1)

            inter = {h: {} for h in range(H)}

            def put(h, kc, u):
                inter[h].setdefault(kc, []).append(u)

            # head 0: V just-in-time + rest of K0
            put(0, 0, lambda: v_proj(0))
            put(0, 0, lambda: v_proj(1))
            for t in range(1, 15):
                put(0, t, lambda t=t: v_proj(t + 1))
            put(0, 5, lambda: k_proj(0, 2))
            put(0, 9, lambda: k_proj(0, 3))
            # head 1: m=1 units (heads 2,3)
            for i in range(4):
                put(1, 2 * i, lambda i=i: k_proj(1, i))
            put(1, 8, lambda: q_proj(1, 0))
            put(1, 10, lambda: q_proj(1, 1))
            # heads 2-3: m=2 units (heads 4,5)
            put(2, 0, lambda: k_proj(2, 0))
            put(2, 5, lambda: k_proj(2, 1))
            put(2, 10, lambda: k_proj(2, 2))
            put(3, 0, lambda: k_proj(2, 3))
            put(3, 5, lambda: q_proj(2, 0))
            put(3, 10, lambda: q_proj(2, 1))
            # heads 4-5: m=3 units (heads 6,7) + out-proj phase A
            put(4, 0, lambda: k_proj(3, 0))
            put(4, 4, lambda: k_proj(3, 1))
            put(4, 5, lambda: k_proj(3, 2))
            put(4, 9, lambda: k_proj(3, 3))
            put(5, 5, lambda: q_proj(3, 0))
            put(5, 10, lambda: q_proj(3, 1))
            for i, kc in enumerate((6, 8, 10, 12, 14)):
                put(4, kc, lambda i=i: phase_a(i))
            for i, kc in enumerate((0, 2, 4)):
                put(5, kc, lambda i=i: phase_a(5 + i))
            # phase B in head 6
            for i, kc in enumerate(range(5, 13)):
                put(6, kc, lambda i=i: phase_b(i))
            # deferred normalization of the previous head's two slots
            for h in range(1, 8):
                put(h, 1, lambda s=2 * (h - 1): norm_slot(s))
                put(h, 3, lambda s=2 * (h - 1) + 1: norm_slot(s))

            # ---------------- attention ----------------
            for h in range(H):
                po = (h % 2) * 64
                chn = h // 2
                cx0 = ps_ctx.tile([65, D], F32, tag="cx")
                cx1 = ps_ctx.tile([65, D], F32, tag="cx")
                for kc in range(16):
                    for u in inter[h].get(kc, []):
                        u()
                    st = ps_st.tile([P, TQ], F32, tag="st")
                    dpo = 64 - po
                    nc.tensor.matmul(
                        st[:, 0:D],
                        lhsT=kt_all[po:po + 64, chn, kc * P:(kc + 1) * P],
                        rhs=qt_all[po:po + 64, chn, 0:D],
                        start=True,
                        stop=True,
                    )
                    if h == 0 and kc < 4:
                        # ramp: avoid the dup-copy dependency so the exp
                        # stream starts as soon as K0/Q0 land
                        nc.tensor.matmul(
                            st[:, D:TQ],
                            lhsT=kt_all[po:po + 64, chn, kc * P:(kc + 1) * P],
                            rhs=qt_all[po:po + 64, chn, D:TQ],
                            start=True,
                            stop=True,
                        )
                    else:
                        nc.tensor.matmul(
                            st[:, D:TQ],
                            lhsT=kt_dup[dpo:dpo + 64, chn, kc * P:(kc + 1) * P],
                            rhs=qt_dup[dpo:dpo + 64, chn, D:TQ],
                            start=True,
                            stop=True,
                        )
                    pr = work.tile([P, TQ], BF16, tag="probs")
                    nc.scalar.activation(
                        out=pr[:, :], in_=st[:, :], func=AF.Exp, scale=0.125
                    )
                    vh = vaug[:, kc, h * 65:(h + 1) * 65]
                    nc.tensor.matmul(
                        cx0[:, :], lhsT=vh, rhs=pr[:, 0:D],
                        start=(kc == 0), stop=(kc == 15),
                    )
                    nc.tensor.matmul(
                        cx1[:, :], lhsT=vh, rhs=pr[:, D:TQ],
                        start=(kc == 0), stop=(kc == 15),
                    )
                # stash raw ctx (+ den row 64), gather den to partition 0
                # (casting DMA), invert in place
                for qt2, cx in ((0, cx0), (1, cx1)):
                    s = 2 * h + qt2
                    nc.vector.tensor_copy(rawc[0:65, s, :], cx[0:65, :])
                    nc.gpsimd.dma_start(
                        out=den_rec[0:1, s % 4, :], in_=rawc[64:65, s, :]
                    )
                    nc.vector.reciprocal_approx_fast(
                        den_rec[0:1, s % 4, :], den_rec[0:1, s % 4, :]
                    )

            # ---------------- tail ----------------
            norm_slot(14)
            norm_slot(15)
            for t8 in range(8):
                phase_c(t8)

    nc.compile()
    return nc


def _get_nc():
    global _NC_CACHE
    if _NC_CACHE is None:
        _NC_CACHE = _build()
    return _NC_CACHE


def _prep_in_maps(x, Wq, bq, Wk, bk, Wv, bv, Wo, bo):
    bf = ml_dtypes.bfloat16
    x = np.asarray(x, np.float32)
    bo = np.asarray(bo, np.float32)
    wqt_n = np.ascontiguousarray(np.asarray(Wq, np.float32).T).astype(bf)
    wkt_n = np.ascontiguousarray(np.asarray(Wk, np.float32).T).astype(bf)
    wvt_n = np.ascontiguousarray(np.asarray(Wv, np.float32).T).astype(bf)
    wot_n = np.ascontiguousarray(np.asarray(Wo, np.float32).T).astype(bf)
    bq_n = np.ascontiguousarray(np.asarray(bq, np.float32).reshape(4, P).T)
    bk_n = np.ascontiguousarray(np.asarray(bk, np.float32).reshape(4, P).T)
    bv_n = np.ascontiguousarray(
        np.broadcast_to(np.asarray(bv, np.float32)[None, :], (P, D))).astype(bf)

    in_maps = []
    for c in range(NCORES):
        b = c // 2
        par = c % 2
        xb = x[b]                               # [S, D]
        xloc = xb[par * TQ:(par + 1) * TQ]      # [TQ, D]
        xoth = xb[(1 - par) * TQ:(2 - par) * TQ]
        xperm = np.concatenate([xloc, xoth], axis=0)   # local queries first
        xr = (xloc + bo[None, :]).reshape(8, P, D).transpose(1, 0, 2)
        xres_n = np.ascontiguousarray(xr.reshape(P, 8 * D)).astype(bf)
        in_maps.append({
            "xt": np.ascontiguousarray(xperm.T).astype(bf),
            "xres": xres_n,
            "wqt": wqt_n, "wkt": wkt_n, "wvt": wvt_n, "wot": wot_n,
            "bq": bq_n, "bk": bk_n, "bv": bv_n,
        })
    return in_maps


def kernel(x, Wq, bq, Wk, bk, Wv, bv, Wo, bo, gamma, beta):
    global LAST_RESULTS
    in_maps = _prep_in_maps(x, Wq, bq, Wk, bk, Wv, bv, Wo, bo)

    nc = _get_nc()
    res = run_bass_kernel_spmd(nc, in_maps, core_ids=list(range(NCORES)), trace=TRACE)
    LAST_RESULTS = res

    outf = np.empty((B, S, D), np.float32)
    for c in range(NCORES):
        b = c // 2
        par = c % 2
        o = np.asarray(res.results[c]["out"], dtype=np.float32)
        o = o.reshape(P, 8, D).transpose(1, 0, 2).reshape(TQ, D)
        outf[b, par * TQ:(par + 1) * TQ, :] = o
    gm = np.asarray(gamma, np.float32)[None, None, :]
    bt = np.asarray(beta, np.float32)[None, None, :]
    return outf * gm + bt
